# revision 1
# baseline (speedup 1.0000x reference)
"""Trainium2 Bass kernel for nn_Attention_16612933501287.

Cross-attention block: c:(B=8,N=8,C=512,H=32,W=32), RMSNorm over C, fused
KV projection (512->1024), one query per (batch, head) attending over the
N=8 token axis at each spatial position, then output projection (512->512).

Sharding: data-parallel over B — one batch element per NeuronCore (8 cores).

Per-core dataflow (feature-major: channels on partitions, the 1024 spatial
positions on the free dim):
  host prep : fold g into Wkv; qv = emb[q]@Wq+bq; fold qv and the 1/sqrt(64)
              logit scale into a per-batch matrix Wd (512x8) so attention
              logits come straight out of a matmul; k is never materialized.
  n loop    : DMA c[n]; square (DVE/ACT/GPSIMD); ssq and logits accumulate
              across n into persistent PSUM tiles via one-hot-padded
              stationary weights; vraw = Wv.T@cp -> fp16 in SBUF.
  epilogue  : batched softmax (one Sqrt + one Exp -> only 2 ACT table
              loads); softmax denominator via an exact-fp32 selection
              matmul; w~ = e*r/sums in fp16; per-head replication via
              broadcast DMAs from a DRAM bounce (all issued upfront);
              vw = vraw*w~ (DVE fp16); sum over n via identity-matmul
              PSUM accumulation; output projection + bias; DMA out in
              (C,H,W) layout.
Big matmuls run as float32r (fp32 data, 1 PE cycle/row).
"""

import numpy as np

import concourse.bass as bass
import concourse.bacc as bacc
import concourse.mybir as mybir
import concourse.tile as tile
from concourse.bass_utils import run_bass_kernel_spmd

F32 = mybir.dt.float32
F16 = mybir.dt.float16
F32R = mybir.dt.float32r
AF = mybir.ActivationFunctionType

B, N, C, H, W = 8, 8, 512, 32, 32
NH, HS = 8, 64
P = H * W           # 1024 spatial positions per core
NCC = C // 128      # 4 contraction chunks
EPS = 1e-6


def r32(ap):
    return ap if ap.dtype == F32R else ap.bitcast(F32R)


def build_program():
    nc = bacc.Bacc()

    c_d = nc.declare_dram_parameter("c", [N, C, H, W], F32R, isOutput=False)
    wv_d = nc.declare_dram_parameter("wv", [128, NCC, 512], F32R, isOutput=False)
    # zero-padded logit weights: [k, cc, n, n*8+i] nonzero only at column n*8+i
    wdz_d = nc.declare_dram_parameter("wdz", [128, NCC, N, N * NH], F32R,
                                      isOutput=False)
    oh_d = nc.declare_dram_parameter("onehot", [128, N, N], F32R, isOutput=False)
    sel_d = nc.declare_dram_parameter("sel", [N * NH, NH], F32, isOutput=False)
    r8_d = nc.declare_dram_parameter("r8sel", [NH, 2, NH * NH], F32, isOutput=False)
    s64_d = nc.declare_dram_parameter("sel64", [N * NH, N, NCC, 128], F16,
                                      isOutput=False)
    wo_d = nc.declare_dram_parameter("wout", [128, NCC, 512], F16, isOutput=False)
    id_d = nc.declare_dram_parameter("ident", [128, 128], F16, isOutput=False)
    bo_d = nc.declare_dram_parameter("bout", [128, NCC], F32, isOutput=False)
    out_d = nc.declare_dram_parameter("out", [C, H, W], F32, isOutput=True)

    with tile.TileContext(nc) as tc:
        with (
            tc.tile_pool(name="consts", bufs=1) as consts,
            tc.tile_pool(name="store", bufs=1) as store,
            tc.tile_pool(name="smalls", bufs=1) as smalls,
            tc.tile_pool(name="osb_pool", bufs=2) as osb_pool,
            tc.tile_pool(name="ps_stat", bufs=1, space="PSUM") as ps_stat,
            tc.tile_pool(name="ps_big", bufs=2, space="PSUM") as ps_big,
        ):
            # loop-critical consts first (tiny oh so PE can start early);
            # wv/wdz loads are emitted inside n=0 after the first cp chunks,
            # epilogue-only weights after the loop.
            wdz_sb = consts.tile([128, NCC, N, N * NH], F32R)
            nc.sync.dma_start(out=wdz_sb[:, 0], in_=wdz_d[:, 0])
            wv_sb = consts.tile([128, NCC, 512], F32R)
            nc.sync.dma_start(out=wv_sb[:, 0], in_=wv_d[:, 0])
            oh_sb = consts.tile([128, N, N], F32R)
            nc.sync.dma_start(out=oh_sb, in_=oh_d[:])
            sel_sb = consts.tile([N * NH, NH], F32)
            r8_sb = consts.tile([NH, 2, NH * NH], F32)
            s64_sb = consts.tile([N * NH, N, NCC, 128], F16)
            wo_sb = consts.tile([128, NCC, 512], F16)
            id_sb = consts.tile([128, 128], F16)
            bo_sb = consts.tile([128, NCC], F32)

            # persistent accumulators / stores
            vraw_all = store.tile([128, N, NCC, P], F16)   # 8 MiB
            o_sb = store.tile([128, NCC, P], F16)
            ssq_ps = ps_stat.tile([N, P], F32)             # 2 banks, whole loop
            draw_ps = ps_stat.tile([N * NH, P], F32)       # 2 banks, whole loop

            # ================= main loop over token index n =================
            cp_ctx = tc.tile_pool(name="cp_pool", bufs=3)
            cp_pool = cp_ctx.__enter__()
            sq_ctx = tc.tile_pool(name="sq_pool", bufs=1)
            sq_pool = sq_ctx.__enter__()
            for n in range(N):
                cp = cp_pool.tile([128, NCC, P], F32R)
                if n == 0:
                    # per-cc loads interleaved with the weights they unblock
                    for cc in range(NCC):
                        nc.sync.dma_start(
                            out=cp[:, cc, :],
                            in_=c_d[:].rearrange(
                                "n (cc k) h w -> n cc k (h w)", k=128)[n, cc],
                        )
                        if cc < NCC - 1:
                            nc.sync.dma_start(out=wdz_sb[:, cc + 1],
                                              in_=wdz_d[:, cc + 1])
                            nc.sync.dma_start(out=wv_sb[:, cc + 1],
                                              in_=wv_d[:, cc + 1])
                else:
                    nc.sync.dma_start(
                        out=cp,
                        in_=c_d[:].rearrange(
                            "n (cc k) h w -> n k cc (h w)", k=128)[n],
                    )

                def emit_draw(n=n, cp=cp):
                    for cc in range(NCC):
                        for h in range(2):
                            nc.tensor.matmul(
                                draw_ps[:, h * 512:(h + 1) * 512],
                                r32(wdz_sb[:, cc, n, :]),
                                r32(cp[:, cc, h * 512:(h + 1) * 512]),
                                start=(n == 0 and cc == 0),
                                stop=(n == N - 1 and cc == NCC - 1),
                            )

                def emit_vraw(n=n, cp=cp):
                    # cc-outer / h-inner: one weight load serves both halves
                    for ck in range(NCC):
                        v_ps = ps_big.tile([128, P], F32, tag="pair",
                                           name="v_ps")
                        for cc in range(NCC):
                            for h in range(2):
                                nc.tensor.matmul(
                                    v_ps[:, h * 512:(h + 1) * 512],
                                    r32(wv_sb[:, cc, ck * 128:(ck + 1) * 128]),
                                    r32(cp[:, cc, h * 512:(h + 1) * 512]),
                                    start=(cc == 0),
                                    stop=(cc == NCC - 1),
                                )
                        nc.scalar.copy(out=vraw_all[:, n, ck, :], in_=v_ps)

                def emit_ssq(n=n, cp=cp):
                    sq = sq_pool.tile([128, NCC, P], F32R, name="sq")
                    nc.vector.tensor_mul(out=sq[:, 0, :], in0=cp[:, 0, :], in1=cp[:, 0, :])
                    nc.gpsimd.tensor_mul(out=sq[:, 1, :], in0=cp[:, 1, :], in1=cp[:, 1, :])
                    nc.gpsimd.tensor_mul(out=sq[:, 2, :], in0=cp[:, 2, :], in1=cp[:, 2, :])
                    nc.gpsimd.tensor_mul(out=sq[:, 3, :], in0=cp[:, 3, :], in1=cp[:, 3, :])
                    # pre-sum the 4 chunks so ssq needs 2 matmuls/n, not 8
                    sqs = sq_pool.tile([128, P], F32R, name="sqs")
                    nc.vector.tensor_add(out=sqs, in0=sq[:, 0, :], in1=sq[:, 1, :])
                    nc.gpsimd.tensor_add(out=sq[:, 2, :], in0=sq[:, 2, :], in1=sq[:, 3, :])
                    nc.vector.tensor_add(out=sqs, in0=sqs, in1=sq[:, 2, :])
                    for h in range(2):
                        nc.tensor.matmul(
                            ssq_ps[:, h * 512:(h + 1) * 512],
                            r32(oh_sb[:, n, :]),
                            r32(sqs[:, h * 512:(h + 1) * 512]),
                            start=(n == 0),
                            stop=(n == N - 1),
                        )

                if n < N - 2:
                    # stats are epilogue-only: emit them last
                    emit_draw(); emit_vraw(); emit_ssq()
                elif n == N - 2:
                    # defer this vraw until after n=7's stats (loop tail)
                    emit_ssq(); emit_draw()
                    deferred_vraw = emit_vraw
                else:
                    # n=7: stats first, then both deferred vraws — the
                    # softmax chain hides under ~14us of vraw matmuls
                    emit_ssq(); emit_draw()
                    deferred_vraw(); emit_vraw()
            sq_ctx.__exit__(None, None, None)
            cp_ctx.__exit__(None, None, None)

            # ======================== epilogue ========================
            # epilogue-only weights (land during the loop's DMA slack)
            nc.sync.dma_start(out=sel_sb, in_=sel_d[:])
            nc.sync.dma_start(out=r8_sb, in_=r8_d[:])
            nc.sync.dma_start(out=s64_sb, in_=s64_d[:])
            nc.sync.dma_start(out=wo_sb, in_=wo_d[:])
            nc.sync.dma_start(out=id_sb, in_=id_d[:])
            nc.sync.dma_start(out=bo_sb, in_=bo_d[:])

            # softmax chain, split into independent h-halves so the two
            # halves pipeline through ACT/DVE/PE (halves the serial latency)
            eps_sb = smalls.tile([N, 1], F32)
            nc.vector.memset(eps_sb, EPS)
            rt = smalls.tile([N, P], F32)
            r_all = rt
            rrep = smalls.tile([N * NH, P], F32)
            e_all = smalls.tile([N * NH, P], F32)
            rsum = smalls.tile([NH, P], F32)
            srep = smalls.tile([N * NH, P], F32)
            wt = smalls.tile([N * NH, P], F16)
            for h in range(2):
                hs_ = slice(h * 512, (h + 1) * 512)
                # r = 1/sqrt(ssq/C + eps)
                nc.scalar.activation(out=rt[:, hs_], in_=ssq_ps[:, hs_],
                                     func=AF.Sqrt, scale=1.0 / C, bias=eps_sb)
                nc.vector.reciprocal_approx_fast(out=r_all[:, hs_], in_=rt[:, hs_])
                # rrep[n*8+i] = r_all[n] via selection matmul (exact fp32)
                rr_ps = ps_big.tile([N * NH, 512], F32, tag="pair", name="rr_ps")
                nc.tensor.matmul(rr_ps, r8_sb[:, 0, :], r_all[:, hs_],
                                 start=True, stop=True)
                nc.scalar.copy(out=rrep[:, hs_], in_=rr_ps)
                # dots = draw * r ; e = exp(dots)
                nc.vector.tensor_mul(out=e_all[:, hs_], in0=draw_ps[:, hs_],
                                     in1=rrep[:, hs_])
                nc.scalar.activation(out=e_all[:, hs_], in_=e_all[:, hs_],
                                     func=AF.Exp)
                # softmax denominator (exact-fp32 matmul), reciprocal
                s_ps = ps_big.tile([NH, 512], F32, tag="pair", name="s_ps")
                nc.tensor.matmul(s_ps, sel_sb, e_all[:, hs_],
                                 start=True, stop=True)
                nc.vector.reciprocal_approx_fast(out=rsum[:, hs_], in_=s_ps)
                sr_ps = ps_big.tile([N * NH, 512], F32, tag="pair", name="sr_ps")
                nc.tensor.matmul(sr_ps, r8_sb[:, 1, :], rsum[:, hs_],
                                 start=True, stop=True)
                nc.scalar.copy(out=srep[:, hs_], in_=sr_ps)
                # w~ = e * r / sums  -> fp16
                nc.vector.tensor_mul(out=e_all[:, hs_], in0=e_all[:, hs_],
                                     in1=rrep[:, hs_])
                nc.vector.tensor_mul(out=wt[:, hs_], in0=e_all[:, hs_],
                                     in1=srep[:, hs_])

            with (
                tc.tile_pool(name="wrep_pool", bufs=4) as wrep_pool,
                tc.tile_pool(name="vw_pool", bufs=2) as vw_pool,
            ):
                # o = sum_n vraw * w~rep via identity-matmul PSUM accumulation;
                # per-head replication via selection matmuls from wt (on-chip)
                for ck in range(NCC):
                    # o-accumulator reuses the (now idle) stats PSUM banks so
                    # ps_big's 4 slots stay free for the wrep pipeline
                    on_ps = ps_stat.tile(
                        [128, P], F32, name=f"on_ps_{ck}",
                        tag=("ssq_ps" if ck % 2 == 0 else "draw_ps"))
                    for n in range(N):
                        vw = vw_pool.tile([128, P], F16)
                        wr_ps = ps_big.tile([128, P], F32, tag="pair")
                        for h in range(2):
                            nc.tensor.matmul(
                                wr_ps[:, h * 512:(h + 1) * 512],
                                s64_sb[:, n, ck, :],
                                wt[:, h * 512:(h + 1) * 512],
                                start=True, stop=True)
                        nc.vector.tensor_mul(
                            out=vw, in0=vraw_all[:, n, ck, :], in1=wr_ps)
                        for h in range(2):
                            nc.tensor.matmul(
                                on_ps[:, h * 512:(h + 1) * 512],
                                id_sb,
                                vw[:, h * 512:(h + 1) * 512],
                                start=(n == 0),
                                stop=(n == N - 1),
                            )
                    for h in range(2):
                        nc.scalar.copy(
                            out=o_sb[:, ck, h * 512:(h + 1) * 512],
                            in_=on_ps[:, h * 512:(h + 1) * 512]
                        )

                # out = Wout.T @ o + bout
                for do in range(NCC):
                    ot_sb = osb_pool.tile([128, P], F32)
                    ot_ps = ps_big.tile([128, P], F32, tag="pair")
                    for h in range(2):
                        for di in range(NCC):
                            nc.tensor.matmul(
                                ot_ps[:, h * 512:(h + 1) * 512],
                                wo_sb[:, di, do * 128:(do + 1) * 128],
                                o_sb[:, di, h * 512:(h + 1) * 512],
                                start=(di == 0),
                                stop=(di == NCC - 1),
                            )
                    nc.scalar.activation(
                        out=ot_sb, in_=ot_ps,
                        func=AF.Identity, bias=bo_sb[:, do:do + 1],
                    )
                    nc.sync.dma_start(
                        out=out_d[:].rearrange(
                            "(do k) h w -> do k (h w)", k=128)[do],
                        in_=ot_sb,
                    )

    nc.finalize()
    return nc


_CACHE = {}


def _get_nc():
    if "nc" not in _CACHE:
        _CACHE["nc"] = build_program()
    return _CACHE["nc"]


def _prep_inputs(q, c, emb, Wq, bq, Wkv, Wout, bout, g):
    q = np.asarray(q)
    c = np.asarray(c, dtype=np.float32)
    emb = np.asarray(emb, dtype=np.float32)
    Wq = np.asarray(Wq, dtype=np.float32)
    bq = np.asarray(bq, dtype=np.float32)
    Wkv = np.asarray(Wkv, dtype=np.float32)
    Wout = np.asarray(Wout, dtype=np.float32)
    bout = np.asarray(bout, dtype=np.float32)
    g = np.asarray(g, dtype=np.float32)

    qv = emb[q] @ Wq + bq                                   # (B, 512)
    qvs = qv.reshape(B, NH, HS).astype(np.float32) * np.float32(HS ** -0.5)
    Wkv_g = (g[:, None] * Wkv).astype(np.float32)
    Wk3 = Wkv_g[:, :C].reshape(C, NH, HS)
    Wv = np.ascontiguousarray(Wkv_g[:, C:])                 # (512, 512)
    Wd = np.einsum('chs,bhs->bch', Wk3, qvs).astype(np.float32)  # (B, 512, 8)

    wv_host = np.ascontiguousarray(
        Wv.reshape(NCC, 128, 512).transpose(1, 0, 2))       # [k, cc, dv]
    # zero-padded draw weights: [b, k, cc, n, m] = Wd at m = n*8+i
    wdz = np.zeros((B, 128, NCC, N, N * NH), np.float32)
    wd4 = Wd.reshape(B, NCC, 128, NH).transpose(0, 2, 1, 3)  # [b, k, cc, i]
    for n in range(N):
        wdz[:, :, :, n, n * NH:(n + 1) * NH] = wd4
    wout_host = np.ascontiguousarray(
        Wout.reshape(NCC, 128, 512).transpose(1, 0, 2)).astype(np.float16)
    onehot = np.zeros((128, N, N), np.float32)
    for n in range(N):
        onehot[:, n, n] = 1.0
    sel = np.zeros((N * NH, NH), np.float32)
    for n in range(N):
        for i in range(NH):
            sel[n * NH + i, i] = 1.0
    # r8sel[:, 0]: rrep (out row n*8+i <- r row n); r8sel[:, 1]: srep (<- rsum row i)
    r8sel = np.zeros((NH, 2, NH * NH), np.float32)
    for n in range(N):
        for i in range(NH):
            r8sel[n, 0, n * NH + i] = 1.0
            r8sel[i, 1, n * NH + i] = 1.0
    # sel64[kk, n, ck, m] = 1 iff kk == n*8 + 2*ck + m//64
    sel64 = np.zeros((N * NH, N, NCC, 128), np.float16)
    for n in range(N):
        for ck in range(NCC):
            for j in range(2):
                sel64[n * NH + 2 * ck + j, n, ck, j * 64:(j + 1) * 64] = 1.0
    ident = np.eye(128, dtype=np.float16)
    bout_host = np.ascontiguousarray(bout.reshape(NCC, 128).T)  # [k, do]

    in_maps = []
    for b in range(B):
        in_maps.append({
            "c": np.ascontiguousarray(c[b]),
            "wv": wv_host,
            "wdz": np.ascontiguousarray(wdz[b]),
            "onehot": onehot,
            "sel": sel,
            "r8sel": r8sel,
            "sel64": sel64,
            "wout": wout_host,
            "ident": ident,
            "bout": bout_host,
        })
    return in_maps


def kernel(**inputs) -> np.ndarray:
    nc = _get_nc()
    in_maps = _prep_inputs(**inputs)
    res = run_bass_kernel_spmd(nc, in_maps, list(range(B)))
    return np.stack([res.results[b]["out"] for b in range(B)], axis=0)


if __name__ == "__main__":
    nc = build_program()
    print("program built ok")



# revision 2
# speedup vs baseline: 2.0139x; 2.0139x over previous
"""Trainium2 Bass kernel for nn_Attention_16612933501287 (v2).

Cross-attention block: c:(B=8,N=8,C=512,H=32,W=32), RMSNorm over C, fused
KV projection (512->1024), one query per (batch, head) attending over the
N=8 token axis at each spatial position, then output projection (512->512).

Sharding: data-parallel over B - one batch element per NeuronCore (8 cores).

v2 design (vs v1): all on-device data fp16 (halves DMA + enables DVE 2x),
two spatial half-chunks (H rows 0-15 / 16-31) pipelined so chunk A's
epilogue overlaps chunk B's main loop, softmax folded into log-space
(wt = exp(dots*r - 0.5*ln(ssq/C+eps) - ln(sum))) so the whole epilogue
needs only Ln/Exp/Copy from ONE activation table set, and the per-head
replication + logit/correction sum happens in a single stacked selection
matmul per (ck, n). o-accumulation runs on DVE fp16 adds (not PE identity
matmuls). Per-core PE work ~225k cycles.

Host prep: fold g into Wkv; qv = emb[q]@Wq+bq; fold qv and the 1/sqrt(64)
scale into per-batch logit weights Wd (zero-padded wdz so logits for all
(n, head) accumulate in one PSUM region); k is never materialized.
"""

import numpy as np

import concourse.bass as bass
import concourse.bacc as bacc
import concourse.mybir as mybir
import concourse.tile as tile
from concourse.bass_utils import run_bass_kernel_spmd

F32 = mybir.dt.float32
F16 = mybir.dt.float16
AF = mybir.ActivationFunctionType

B, N, C, H, W = 8, 8, 512, 32, 32
NH, HS = 8, 64
NCC = C // 128      # contraction chunks over input channels
NCK = 4             # output chunks over v channels
NCH = 2             # spatial chunks (H halves)
PC = H * W // NCH   # 512 positions per chunk
EPS = 1e-6


INST_LABELS = {}
_CUR_LABEL = ["init"]


def _label(s):
    _CUR_LABEL[0] = s


def build_program(reps=1):
    nc = bacc.Bacc()
    INST_LABELS.clear()
    _orig_gn = nc.get_next_instruction_name

    def _gn():
        name = _orig_gn()
        INST_LABELS[name] = _CUR_LABEL[0]
        return name
    nc.get_next_instruction_name = _gn

    c_d = nc.declare_dram_parameter("c", [N, C, H, W], F16, isOutput=False)
    wv_d = nc.declare_dram_parameter("wv", [128, NCC, 512], F16, isOutput=False)
    # zero-padded logit weights: [k, cc, n, n*8+i] nonzero only at col n*8+i
    wdz_d = nc.declare_dram_parameter("wdz", [128, NCC, N, N * NH], F16,
                                      isOutput=False)
    oh_d = nc.declare_dram_parameter("onehot", [128, N, N], F16, isOutput=False)
    r8_d = nc.declare_dram_parameter("r8sel", [N, N * NH], F16, isOutput=False)
    s8_d = nc.declare_dram_parameter("sel8", [N * NH, NH], F16, isOutput=False)
    # ln-space softmax corrections: [0]: -0.5*lt[n] sel, [1]: -lns[h] sel
    s16_d = nc.declare_dram_parameter("sel16", [2, N, N * NH], F16,
                                      isOutput=False)
    # per-head replication: out row p of (n, ck) gets wt[n*8 + 2*ck + p//64]
    s64_d = nc.declare_dram_parameter("sel64", [N * NH, N, NCK, 128], F16,
                                      isOutput=False)
    wo_d = nc.declare_dram_parameter("wout", [128, NCC, 512], F16, isOutput=False)
    bo_d = nc.declare_dram_parameter("bout", [128, NCC], F32, isOutput=False)
    out_d = nc.declare_dram_parameter("out", [C, H, W], F16, isOutput=True)

    # DRAM-side access patterns (per chunk ch, token n)
    c_ap = c_d[:].rearrange("n (cc k) (hh hr) w -> hh n k cc (hr w)",
                            cc=NCC, hh=NCH)
    out_ap = out_d[:].rearrange("(do k) (hh hr) w -> hh do k (hr w)",
                                do=NCC, hh=NCH)

    with tile.TileContext(nc) as tc:
        with (
            tc.tile_pool(name="consts", bufs=1) as consts,
            tc.tile_pool(name="vraw_pool", bufs=2) as vraw_pool,
            tc.tile_pool(name="o_pool", bufs=2) as o_pool,
            tc.tile_pool(name="sm_pool", bufs=2) as sm_pool,
            tc.tile_pool(name="cp_pool", bufs=4) as cp_pool,
            tc.tile_pool(name="sq_pool", bufs=2) as sq_pool,
            tc.tile_pool(name="wt_pool", bufs=3) as wt_pool,
            tc.tile_pool(name="vw_pool", bufs=3) as vw_pool,
            tc.tile_pool(name="osb_pool", bufs=2) as osb_pool,
            tc.tile_pool(name="ps_stat", bufs=2, space="PSUM") as ps_stat,
            tc.tile_pool(name="ps_loop", bufs=2, space="PSUM") as ps_loop,
        ):
            rep_ctx = tc.For_i(0, reps, 1) if reps > 1 else None
            if rep_ctx is not None:
                rep_ctx.__enter__()

            # ---- weights: loop-critical first, epilogue weights later ----
            wdz_sb = consts.tile([128, NCC, N, N * NH], F16)
            nc.sync.dma_start(out=wdz_sb, in_=wdz_d[:])
            wv_sb = consts.tile([128, NCC, 512], F16)
            nc.sync.dma_start(out=wv_sb, in_=wv_d[:])
            oh_sb = consts.tile([128, N, N], F16)
            nc.sync.dma_start(out=oh_sb, in_=oh_d[:])
            r8_sb = consts.tile([N, N * NH], F16)
            s8_sb = consts.tile([N * NH, NH], F16)
            s16a_sb = consts.tile([N, N * NH], F16)
            s16b_sb = consts.tile([N, N * NH], F16)
            s64_sb = consts.tile([N * NH, N, NCK, 128], F16)
            wo_sb = consts.tile([128, NCC, 512], F16)
            bo_sb = consts.tile([128, NCC], F32)
            eps_sb = consts.tile([N, 1], F32)
            nc.vector.memset(eps_sb, EPS)

            epi_weights_loaded = [False]

            def load_epi_weights():
                nc.sync.dma_start(out=r8_sb, in_=r8_d[:])
                nc.sync.dma_start(out=s8_sb, in_=s8_d[:])
                nc.sync.dma_start(out=s16a_sb, in_=s16_d[0])
                nc.sync.dma_start(out=s16b_sb, in_=s16_d[1])
                nc.sync.dma_start(out=s64_sb, in_=s64_d[:])
                nc.sync.dma_start(out=wo_sb, in_=wo_d[:])
                nc.sync.dma_start(out=bo_sb, in_=bo_d[:])
                epi_weights_loaded[0] = True

            # per-chunk state carried between emission phases
            state = {}

            def emit_loop_n(ch, n, stats_ps, vraw_all):
                _label(f"loop{ch}.n{n}")
                cp = cp_pool.tile([128, NCC, PC], F16, name="cp")
                nc.sync.dma_start(out=cp, in_=c_ap[ch, n])
                # logits: accumulate over (n, cc) into stats[0:64]
                for cc in range(NCC):
                    nc.tensor.matmul(
                        stats_ps[0:64, :],
                        wdz_sb[:, cc, n, :],
                        cp[:, cc, :],
                        start=(n == 0 and cc == 0),
                        stop=(n == N - 1 and cc == NCC - 1),
                    )
                # v projection: ck-pairs share one PSUM tile so the ACT copy
                # runs at F=1024 (amortizes the fixed access latency)
                for cp_i in range(NCK // 2):
                    v_ps = ps_loop.tile([128, 2, PC], F32, name="v_ps", tag="vbig", bufs=2)
                    for half in range(2):
                        ck = 2 * cp_i + half
                        for cc in range(NCC):
                            nc.tensor.matmul(
                                v_ps[:, half, :],
                                wv_sb[:, cc, ck * 128:(ck + 1) * 128],
                                cp[:, cc, :],
                                start=(cc == 0),
                                stop=(cc == NCC - 1),
                            )
                    nc.scalar.copy(
                        out=vraw_all[:, n, 2 * cp_i:2 * cp_i + 2, :], in_=v_ps)
                # squared sums for RMSNorm: paired squares on ACT + DVE,
                # partial sums on DVE/Pool
                sq = sq_pool.tile([128, NCC, PC], F16, name="sq")
                nc.scalar.activation(out=sq[:, 0:2, :], in_=cp[:, 0:2, :],
                                     func=AF.Square)
                nc.vector.tensor_mul(out=sq[:, 2:4, :], in0=cp[:, 2:4, :],
                                     in1=cp[:, 2:4, :])
                nc.vector.tensor_add(out=sq[:, 0, :], in0=sq[:, 0, :], in1=sq[:, 1, :])
                nc.gpsimd.tensor_add(out=sq[:, 2, :], in0=sq[:, 2, :], in1=sq[:, 3, :])
                nc.vector.tensor_add(out=sq[:, 0, :], in0=sq[:, 0, :], in1=sq[:, 2, :])
                nc.tensor.matmul(
                    stats_ps[64:72, :],
                    oh_sb[:, n, :],
                    sq[:, 0, :],
                    start=(n == 0),
                    stop=(n == N - 1),
                    skip_group_check=True,
                )

            def emit_smax1(ch):
                """lt = ln(ssq/C+eps); r = exp(-0.5 lt)."""
                _label(f"smax{ch}")
                stats_ps = state[ch]["stats"]
                lt_sb = sm_pool.tile([N, PC], F16, name="lt_sb")
                nc.scalar.activation(out=lt_sb, in_=stats_ps[64:72, :],
                                     func=AF.Ln, scale=1.0 / C, bias=eps_sb)
                r_sb = sm_pool.tile([N, PC], F16, name="r_sb")
                nc.scalar.activation(out=r_sb, in_=lt_sb,
                                     func=AF.Exp, scale=-0.5)
                state[ch]["lt"] = lt_sb
                state[ch]["r"] = r_sb

            def emit_smax2(ch):
                """rrep; d~ = draw * rrep."""
                _label(f"smax{ch}")
                stats_ps = state[ch]["stats"]
                rr_ps = ps_loop.tile([128, 2, PC], F32, name="rr_ps",
                                     tag="vbig", bufs=2)[0:N * NH, 0, :]
                nc.tensor.matmul(rr_ps, r8_sb, state[ch]["r"], start=True,
                                 stop=True)
                rrep = sm_pool.tile([N * NH, PC], F16, name="rrep")
                nc.scalar.copy(out=rrep, in_=rr_ps)
                dt_sb = sm_pool.tile([N * NH, PC], F16, name="dt_sb")
                nc.vector.tensor_mul(out=dt_sb, in0=stats_ps[0:64, :], in1=rrep)
                state[ch]["dt"] = dt_sb

            def emit_smax3(ch):
                """e = exp(d~); s = sum_n e; lns = ln(s)."""
                _label(f"smax{ch}")
                e_sb = sm_pool.tile([N * NH, PC], F16, name="e_sb")
                nc.scalar.activation(out=e_sb, in_=state[ch]["dt"], func=AF.Exp)
                s_ps = ps_loop.tile([128, 2, PC], F32, name="s_ps",
                                    tag="vbig", bufs=2)[0:NH, 0, :]
                nc.tensor.matmul(s_ps, s8_sb, e_sb, start=True, stop=True)
                lns_sb = sm_pool.tile([N, PC], F16, name="lns_sb")
                nc.scalar.activation(out=lns_sb, in_=s_ps, func=AF.Ln)
                state[ch]["e"] = e_sb
                state[ch]["lns"] = lns_sb

            def emit_smax4(ch):
                """adj = -0.5 lt - lns (replicated); wt = e * exp(adj)."""
                _label(f"smax{ch}")
                adj_ps = ps_loop.tile([128, 2, PC], F32, name="adj_ps",
                                      tag="vbig", bufs=2)[0:N * NH, 0, :]
                nc.tensor.matmul(adj_ps, s16a_sb, state[ch]["lt"],
                                 start=True, stop=False)
                nc.tensor.matmul(adj_ps, s16b_sb, state[ch]["lns"],
                                 start=False, stop=True)
                ea_sb = sm_pool.tile([N * NH, PC], F16, name="ea_sb")
                nc.scalar.activation(out=ea_sb, in_=adj_ps, func=AF.Exp)
                wt_sb = sm_pool.tile([N * NH, PC], F16, name="wt_sb")
                nc.vector.tensor_mul(out=wt_sb, in0=state[ch]["e"], in1=ea_sb)
                state[ch]["wt"] = wt_sb

            def make_units(ch):
                """32 (ck, n) units: replicate wt rows -> mult (PSUM) -> acc.

                Engine split so the overlap window stays balanced: mults
                ck0-2 on DVE, ck3 on Pool; accumulate chains ck0/1 on DVE,
                ck2/3 on Pool (independent in-place chains per engine)."""
                wt_sb = state[ch]["wt"]
                vraw_all = state[ch]["vraw"]
                o_sb = state[ch]["o"]
                units = []
                for n in range(N):
                    for ck in range(NCK):
                        def unit(ck=ck, n=n):
                            _label(f"unit{ch}.ck{ck}n{n}")
                            mul_eng = nc.vector
                            add_eng = nc.vector if ck == 0 else nc.gpsimd
                            wt_ps = ps_loop.tile(
                                [128, PC], F32, name="wt_ps",
                                tag="wt", bufs=2)
                            nc.tensor.matmul(wt_ps, s64_sb[:, n, ck, :],
                                             wt_sb, start=True, stop=True)
                            if n == 0:
                                mul_eng.tensor_mul(
                                    out=o_sb[:, ck, :],
                                    in0=vraw_all[:, n, ck, :], in1=wt_ps)
                            else:
                                vw = vw_pool.tile(
                                    [128, PC], F16,
                                    name=("vw_d" if add_eng is nc.vector
                                          else "vw_p"))
                                mul_eng.tensor_mul(
                                    out=vw, in0=vraw_all[:, n, ck, :],
                                    in1=wt_ps)
                                add_eng.tensor_add(
                                    out=o_sb[:, ck, :],
                                    in0=o_sb[:, ck, :], in1=vw)
                        units.append(unit)
                return units

            def emit_wout(ch):
                _label(f"wout{ch}")
                o_sb = state[ch]["o"]
                for do in range(NCC):
                    ot_ps = ps_loop.tile([128, PC], F32, name="ot_ps", tag="wt", bufs=2)
                    for di in range(NCC):
                        nc.tensor.matmul(
                            ot_ps,
                            wo_sb[:, di, do * 128:(do + 1) * 128],
                            o_sb[:, di, :],
                            start=(di == 0),
                            stop=(di == NCC - 1),
                        )
                    ot_sb = osb_pool.tile([128, PC], F16)
                    nc.scalar.activation(out=ot_sb, in_=ot_ps,
                                         func=AF.Identity,
                                         bias=bo_sb[:, do:do + 1])
                    nc.sync.dma_start(out=out_ap[ch, do], in_=ot_sb)

            UNIT_TAKE = {3: 7, 4: 7, 5: 6, 6: 6, 7: 6}
            prev_ch = None
            for ch in range(NCH):
                stats_ps = ps_stat.tile([72, PC], F32, name="stats")
                vraw_all = vraw_pool.tile([128, N, NCK, PC], F16,
                                          name="vraw")
                o_sb = o_pool.tile([128, NCC, PC], F16, name="o_sb")
                state[ch] = {"stats": stats_ps, "vraw": vraw_all, "o": o_sb}
                prev_units = None
                for n in range(N):
                    emit_loop_n(ch, n, stats_ps, vraw_all)
                    if n == 0 and not epi_weights_loaded[0]:
                        load_epi_weights()
                    # interleave previous chunk's softmax chain + units
                    if prev_ch is not None:
                        if n == 0:
                            emit_smax1(prev_ch)
                        elif n == 1:
                            emit_smax2(prev_ch)
                        elif n == 2:
                            emit_smax3(prev_ch)
                        elif n == 3:
                            emit_smax4(prev_ch)
                            prev_units = make_units(prev_ch)
                        if prev_units:
                            for _ in range(UNIT_TAKE[n]):
                                if prev_units:
                                    prev_units.pop(0)()
                if prev_ch is not None:
                    emit_wout(prev_ch)
                prev_ch = ch

            # tail: last chunk's full epilogue
            emit_smax1(prev_ch)
            emit_smax2(prev_ch)
            emit_smax3(prev_ch)
            emit_smax4(prev_ch)
            for u in make_units(prev_ch):
                u()
            emit_wout(prev_ch)

            if rep_ctx is not None:
                rep_ctx.__exit__(None, None, None)

    nc.finalize()
    return nc


_CACHE = {}


def _get_nc():
    if "nc" not in _CACHE:
        _CACHE["nc"] = build_program()
    return _CACHE["nc"]


def _prep_inputs(q, c, emb, Wq, bq, Wkv, Wout, bout, g):
    q = np.asarray(q)
    c = np.asarray(c, dtype=np.float32)
    emb = np.asarray(emb, dtype=np.float32)
    Wq = np.asarray(Wq, dtype=np.float32)
    bq = np.asarray(bq, dtype=np.float32)
    Wkv = np.asarray(Wkv, dtype=np.float32)
    Wout = np.asarray(Wout, dtype=np.float32)
    bout = np.asarray(bout, dtype=np.float32)
    g = np.asarray(g, dtype=np.float32)

    qv = emb[q] @ Wq + bq                                   # (B, 512)
    qvs = qv.reshape(B, NH, HS).astype(np.float32) * np.float32(HS ** -0.5)
    Wkv_g = (g[:, None] * Wkv).astype(np.float32)
    Wk3 = Wkv_g[:, :C].reshape(C, NH, HS)
    Wv = np.ascontiguousarray(Wkv_g[:, C:])                 # (512, 512)
    Wd = np.einsum('chs,bhs->bch', Wk3, qvs).astype(np.float32)  # (B, 512, 8)

    wv_host = np.ascontiguousarray(
        Wv.reshape(NCC, 128, 512).transpose(1, 0, 2)).astype(np.float16)
    # zero-padded draw weights: [b, k, cc, n, m] = Wd at m = n*8+i
    wdz = np.zeros((B, 128, NCC, N, N * NH), np.float16)
    wd4 = Wd.reshape(B, NCC, 128, NH).transpose(0, 2, 1, 3)  # [b, k, cc, i]
    for n in range(N):
        wdz[:, :, :, n, n * NH:(n + 1) * NH] = wd4
    onehot = np.zeros((128, N, N), np.float16)
    for n in range(N):
        onehot[:, n, n] = 1.0
    r8 = np.zeros((N, N * NH), np.float16)
    s8 = np.zeros((N * NH, NH), np.float16)
    for n in range(N):
        for i in range(NH):
            r8[n, n * NH + i] = 1.0
            s8[n * NH + i, i] = 1.0
    # adj[n*8+h] = -0.5*lt[n] - lns[h], two stationary blocks
    s16 = np.zeros((2, N, N * NH), np.float16)
    for n in range(N):
        for i in range(NH):
            s16[0, n, n * NH + i] = -0.5
            s16[1, i, n * NH + i] = -1.0
    # per-head replication: out row p of (n, ck) <- wt row n*8 + 2*ck + p//64
    s64 = np.zeros((N * NH, N, NCK, 128), np.float16)
    for n in range(N):
        for ck in range(NCK):
            for p in range(128):
                s64[n * NH + 2 * ck + p // 64, n, ck, p] = 1.0
    wout_host = np.ascontiguousarray(
        Wout.reshape(NCC, 128, 512).transpose(1, 0, 2)).astype(np.float16)
    bout_host = np.ascontiguousarray(bout.reshape(NCC, 128).T)  # [k, do]

    in_maps = []
    for b in range(B):
        in_maps.append({
            "c": np.ascontiguousarray(c[b]).astype(np.float16),
            "wv": wv_host,
            "wdz": np.ascontiguousarray(wdz[b]),
            "onehot": onehot,
            "r8sel": r8,
            "sel8": s8,
            "sel16": s16,
            "sel64": s64,
            "wout": wout_host,
            "bout": bout_host,
        })
    return in_maps


def kernel(**inputs) -> np.ndarray:
    nc = _get_nc()
    in_maps = _prep_inputs(**inputs)
    res = run_bass_kernel_spmd(nc, in_maps, list(range(B)))
    return np.stack(
        [res.results[b]["out"].astype(np.float32) for b in range(B)], axis=0)


if __name__ == "__main__":
    nc = build_program()
    print("program built ok")
